# revision 62
# baseline (speedup 1.0000x reference)
"""Graph-transformer block on 8 Trainium2 NeuronCores.

Sharding: each core takes a 512-row q-slice of the 4096 nodes across ALL 4
heads. No cross-core communication: each core computes attention for its
q rows, runs the FFN on its node slice, writes its [512, 256] output slice.

Per-core pipeline (transposed-S orientation — no on-device transposes):
  prep:  q/k/v projections as fp8 DoubleRow matmuls (h and Wq/Wk/Wv ship
         as fp8; the IN=256 contraction runs as 2 k-tiles of 128), copied
         out of PSUM to bf16 qT/kT (head pairs packed on partitions) and
         to v with an interleaved all-ones column per head (the softmax
         denominator rides the PV matmul as output row 64). PSUM->SBUF
         copies go through the Scalar engine (ACT); GPSIMD cannot read
         PSUM. Prep chunks past j-block 7 are emitted at the END of
         attention loop bodies so they fill PE slack instead of blocking
         the S^T chain.
  attn:  per 128-wide j-block jb:
           one fp8 DMA of adjT for all 4 heads [128j, 4*512q]
           per head pair g: two S^T bf16 matmuls into one [128, 2, 512]
             PSUM tile (head 2g+i from partitions i*64 of packed kT/qT)
           mk = (S^T * 1/16) * adjT   (DVE STT, PSUM fp32 * fp8 -> bf16)
           P^T = exp(mk) on ACT, one [128, 2048] activation per j-block
           xaug[hd] [65, 512] += v_aug_blk.T @ P^T on PE (PSUM accum)
         PV matmuls lag the S^T stage by PIPE j-blocks so the in-order PE
         stream never waits on the DVE->ACT chain.
  fin:   embT = xaug[0:64] * recip(xaug[64]) broadcast -> [256, 512] bf16
  ffn:   p1^T = relu(W1.T @ embT + b1); p2 = p1 @ W2 (+ b2 via rank-1 ones
         matmul); row softmax over 256 features (logits are ~1e-3 scale so
         exp is safe without max-subtraction); DMA out fp32

Measured on hw: 131232 ns vs the 1368737 ns session baseline (10.4x).
Rejected by measurement (slower on hardware despite cost-model wins):
fp8 DoubleRow for the S^T or PV attention matmuls, fp8 S^T operands,
SBUF->SBUF DMA transposes of P (the original baseline's approach),
prep fully upfront, deeper PV lag.
"""
import sys
import numpy as np

sys.path.insert(0, "/opt/trn_rl_repo")
import ml_dtypes  # noqa: E402

IN = 256
H = 4
DH = 64
NCORES = 8
F1 = 512
DOUT = 256
N_NODES = 4096
QS = 512
NJB = N_NODES // 128
SCALE = 1.0 / 16.0  # 1/sqrt(IN)
PIPE = 2            # PV matmuls lag S^T by this many j-blocks

_cache = {}


def build():
    if "nc" in _cache:
        return _cache["nc"]

    from contextlib import ExitStack
    import concourse.tile as tile
    from concourse import mybir, bacc
    from concourse.alu_op_type import AluOpType

    fp32, bf16 = mybir.dt.float32, mybir.dt.bfloat16
    fp8 = mybir.dt.float8e4
    AF = mybir.ActivationFunctionType
    MUL = AluOpType.mult
    ADD = AluOpType.add

    nc = bacc.Bacc("TRN2", target_bir_lowering=False, debug=False,
                   enable_asserts=False)

    adjt_d = nc.dram_tensor("adjt", [N_NODES, H * QS], fp8, kind="ExternalInput").ap()
    ht8_d = nc.dram_tensor("ht8", [128, 2, N_NODES], fp8, kind="ExternalInput").ap()
    htq8_d = nc.dram_tensor("htq8", [128, 2, QS], fp8, kind="ExternalInput").ap()
    wq8_d = nc.dram_tensor("wq8", [128, 2, IN], fp8, kind="ExternalInput").ap()
    wk8_d = nc.dram_tensor("wk8", [128, 2, IN], fp8, kind="ExternalInput").ap()
    wv8_d = nc.dram_tensor("wv8", [128, 2, IN], fp8, kind="ExternalInput").ap()
    w1_d = nc.dram_tensor("w1", [IN, F1], bf16, kind="ExternalInput").ap()
    w2_d = nc.dram_tensor("w2", [F1, DOUT], bf16, kind="ExternalInput").ap()
    b1_d = nc.dram_tensor("b1", [128, F1 // 128], fp32, kind="ExternalInput").ap()
    b2b_d = nc.dram_tensor("b2b", [1, DOUT], bf16, kind="ExternalInput").ap()
    out_d = nc.dram_tensor("out", [QS, DOUT], fp32, kind="ExternalOutput").ap()

    with ExitStack() as ctx:
        tc = ctx.enter_context(tile.TileContext(nc))
        pc = ctx.enter_context(tc.tile_pool(name="const", bufs=1))
        pst = ctx.enter_context(tc.tile_pool(name="stp", bufs=2, space="PSUM"))
        pxt = ctx.enter_context(tc.tile_pool(name="xtp", bufs=1, space="PSUM"))
        pa = ctx.enter_context(tc.tile_pool(name="adjp", bufs=6))
        pm = ctx.enter_context(tc.tile_pool(name="mkp", bufs=5))
        ppt = ctx.enter_context(tc.tile_pool(name="ptp", bufs=4))
        psm = ctx.enter_context(tc.tile_pool(name="smallp", bufs=2))

        # ---------------- constant loads ----------------
        # hTq + weights + the first h j-chunk land first so the first prep
        # matmuls aren't gated on the full h transfer.
        # q-path loads go on the otherwise-idle Sync queue so they land in
        # parallel with the gpsimd-issued h/weight loads (earlier first S^T).
        htq8_sb = pc.tile([128, 2, QS], fp8, tag="htq8")
        nc.sync.dma_start(out=htq8_sb[:, :, :], in_=htq8_d[:, :, :])
        wq8_sb = pc.tile([128, 2, IN], fp8, tag="wq8")
        wk8_sb = pc.tile([128, 2, IN], fp8, tag="wk8")
        wv8_sb = pc.tile([128, 2, IN], fp8, tag="wv8")
        nc.sync.dma_start(out=wq8_sb[:, :, :], in_=wq8_d[:, :, :])
        nc.sync.dma_start(out=wk8_sb[:, :, :], in_=wk8_d[:, :, :])
        nc.gpsimd.dma_start(out=wv8_sb[:, :, :], in_=wv8_d[:, :, :])
        ht8_sb = pc.tile([128, 2, N_NODES], fp8, tag="ht8")
        for jt in range(4):
            nc.gpsimd.dma_start(
                out=ht8_sb[:, :, jt * 1024:(jt + 1) * 1024],
                in_=ht8_d[:, :, jt * 1024:(jt + 1) * 1024])
        w1_sb = [pc.tile([128, F1], bf16, tag=f"w1_{dc}", name=f"w1_{dc}") for dc in range(2)]
        for dc in range(2):
            nc.gpsimd.dma_start(out=w1_sb[dc][:], in_=w1_d[dc * 128:(dc + 1) * 128, :])
        w2_sb = pc.tile([128, 4 * DOUT], bf16, tag="w2")
        for fc in range(4):
            nc.gpsimd.dma_start(out=w2_sb[:, fc * DOUT:(fc + 1) * DOUT],
                                in_=w2_d[fc * 128:(fc + 1) * 128, :])
        b1_sb = pc.tile([128, F1 // 128], fp32, tag="b1")
        nc.gpsimd.dma_start(out=b1_sb[:], in_=b1_d[:, :])
        b2b_sb = pc.tile([1, DOUT], bf16, tag="b2b")
        nc.gpsimd.dma_start(out=b2b_sb[:], in_=b2b_d[:, :])
        ones1_sb = pc.tile([1, 128], bf16, tag="ones1")
        nc.gpsimd.memset(ones1_sb[:], 1.0)

        # ---------------- projections ----------------
        # qT/kT bf16, head pairs packed on partitions (pair p -> head 2p at
        # partitions 0-63, 2p+1 at 64-127).
        qT_sb = [pc.tile([128, QS], bf16, tag=f"qT{p}", name=f"qT{p}") for p in range(2)]
        kT_sb = [pc.tile([128, N_NODES], bf16, tag=f"kT{p}", name=f"kT{p}") for p in range(2)]
        # v with interleaved ones columns: vp[:, jb*4+hd, 0:64] = v values for
        # head hd at j-block jb; vp[:, *, 64] = 1.0 (softmax denominator row)
        vp = pc.tile([128, NJB * H, DH + 1], bf16, tag="vp")
        nc.vector.memset(vp[:, :, DH:DH + 1], 1.0)

        DR = mybir.MatmulPerfMode.DoubleRow

        def emit_qT(p):
            ps = pst.tile([128, QS], fp32, tag="st", name=f"qTps{p}")
            nc.tensor.matmul(ps[:], wq8_sb[:, :, p * 128:(p + 1) * 128],
                             htq8_sb[:, :, :], start=True, stop=True,
                             perf_mode=DR)
            nc.scalar.copy(qT_sb[p][:], ps[:])

        def emit_kT(p, jt):  # one 1024-wide j chunk of kT for pair p
            ps = pst.tile([128, 2, 512], fp32, tag="st", name=f"kTps{p}_{jt}")
            for half in range(2):
                nc.tensor.matmul(
                    ps[:, half, :],
                    wk8_sb[:, :, p * 128:(p + 1) * 128],
                    ht8_sb[:, :, jt * 1024 + half * 512: jt * 1024 + (half + 1) * 512],
                    start=True, stop=True, perf_mode=DR)
            nc.scalar.copy(kT_sb[p][:, jt * 1024:(jt + 1) * 1024], ps[:, :, :])

        def emit_v(jq):  # v (with interleaved ones) for j-blocks 4jq..4jq+3
            ps = pst.tile([128, 16, DH], fp32, tag="st", name=f"vps{jq}")
            for jj in range(4):
                jb = jq * 4 + jj
                nc.tensor.matmul(ps[:, jj * 4:(jj + 1) * 4, :],
                                 ht8_sb[:, :, jb * 128:(jb + 1) * 128],
                                 wv8_sb[:, :, :],
                                 start=True, stop=True, perf_mode=DR)
            nc.scalar.copy(vp[:, jq * 16:(jq + 1) * 16, 0:DH], ps[:, :, :])

        emit_qT(0)
        emit_qT(1)
        emit_kT(0, 0)
        emit_kT(1, 0)
        emit_v(0)
        emit_v(1)
        # remaining chunks ordered by deadline: kT jt=t needed at jb 8t,
        # v quad jq needed at jb 4jq+2 (PV lags by PIPE).
        prep_chunks = []
        for t in range(1, 4):
            prep_chunks.append(lambda jt=t, p=0: emit_kT(p, jt))
            prep_chunks.append(lambda jt=t, p=1: emit_kT(p, jt))
            prep_chunks.append(lambda jq=2 * t: emit_v(jq))
            prep_chunks.append(lambda jq=2 * t + 1: emit_v(jq))

        # ---------------- attention ----------------
        embT_sb = [pc.tile([128, QS], bf16, tag=f"embT{p}", name=f"embT{p}") for p in range(2)]
        xaug = [pxt.tile([DH + 1, QS], fp32, tag=f"xt{hd}", name=f"xt{hd}") for hd in range(H)]

        pt_q = []
        for jb in range(NJB + PIPE):
            if jb < NJB:
                aj = pa.tile([128, H * QS], fp8, tag="aj")
                nc.sync.dma_start(out=aj[:],
                                  in_=adjt_d[jb * 128:(jb + 1) * 128, :])
                mk4 = pm.tile([128, H, QS], bf16, tag="mk")
                pt4 = ppt.tile([128, H, QS], bf16, tag="pt")
                for g in range(2):  # head pair
                    st2 = pst.tile([128, 2, 512], fp32, tag="st")
                    for i in range(2):  # head 2g+i from partitions i*64
                        nc.tensor.matmul(
                            st2[:, i, :],
                            kT_sb[g][i * 64:(i + 1) * 64, jb * 128:(jb + 1) * 128],
                            qT_sb[g][i * 64:(i + 1) * 64, :],
                            start=True, stop=True)
                    nc.vector.scalar_tensor_tensor(
                        mk4[:, 2 * g:2 * g + 2, :], st2[:, :, :], SCALE,
                        aj[:, g * 1024:(g + 1) * 1024], MUL, MUL)
                nc.scalar.activation(pt4[:, :, :], mk4[:, :, :], AF.Exp)
                pt_q.append((jb, pt4))
            if jb >= PIPE:
                j2, pt = pt_q.pop(0)
                for hd in range(H):
                    nc.tensor.matmul(xaug[hd][:],
                                     vp[:, j2 * H + hd, 0:DH + 1],
                                     pt[:, hd, :],
                                     start=(j2 == 0), stop=(j2 == NJB - 1))
            # prep chunks fill PE slack behind the attention chain
            if jb % 2 == 0 and prep_chunks:
                prep_chunks.pop(0)()

        # epilogue: embT = xaug[0:64] * (1/denom), denom = xaug row 64.
        # Stage-major emission so the four heads pipeline across DVE/Pool.
        dens = []
        for hd in range(H):
            den = psm.tile([1, QS], fp32, tag=f"den{hd}", name=f"den{hd}")
            nc.vector.tensor_copy(den[:], xaug[hd][DH:DH + 1, :])
            dens.append(den)
        rcps = []
        for hd in range(H):
            rcp = psm.tile([1, QS], fp32, tag=f"rcp{hd}", name=f"rcp{hd}")
            nc.vector.reciprocal_approx_fast(rcp[:], dens[hd][:])
            rcps.append(rcp)
        rbcs = []
        for hd in range(H):
            rbc = psm.tile([128, QS], fp32, tag=f"rbc{hd}", name=f"rbc{hd}")
            nc.gpsimd.partition_broadcast(rbc[:], rcps[hd][0:1, :])
            rbcs.append(rbc)
        for hd in range(H):
            p, off = hd // 2, (hd % 2) * 64
            nc.vector.tensor_tensor(embT_sb[p][off:off + 64, :],
                                    xaug[hd][0:DH, :], rbcs[hd][off:off + 64, :],
                                    MUL)

        # ---------------- FFN + row softmax ----------------
        p1_sb = pc.tile([128, F1 // 128, QS], bf16, tag="p1")
        for fc in range(F1 // 128):
            ps = pst.tile([128, QS], fp32, tag="st")
            for dc in range(2):
                nc.tensor.matmul(ps[:], w1_sb[dc][:, fc * 128:(fc + 1) * 128],
                                 embT_sb[dc][:], start=(dc == 0), stop=(dc == 1))
            nc.scalar.activation(p1_sb[:, fc, :], ps[:], AF.Relu,
                                 bias=b1_sb[:, fc:fc + 1])
        for qc in range(QS // 128):
            ps2 = pst.tile([128, DOUT], fp32, tag="st")
            for fc in range(F1 // 128):
                nc.tensor.matmul(ps2[:],
                                 p1_sb[:, fc, qc * 128:(qc + 1) * 128],
                                 w2_sb[:, fc * DOUT:(fc + 1) * DOUT],
                                 start=(fc == 0), stop=False)
            nc.tensor.matmul(ps2[:], ones1_sb[0:1, :], b2b_sb[0:1, :],
                             start=False, stop=True)
            # logits are ~1e-2 scale here, so exp() is overflow-safe without
            # the usual max-subtraction (softmax is shift-invariant).
            e = psm.tile([128, DOUT], fp32, tag="e")
            sm = psm.tile([128, 1], fp32, tag="sm")
            nc.scalar.activation(e[:], ps2[:], AF.Exp, accum_out=sm[:])
            rc = psm.tile([128, 1], fp32, tag="rc")
            nc.vector.reciprocal_approx_fast(rc[:], sm[:])
            o = psm.tile([128, DOUT], fp32, tag="o")
            nc.vector.tensor_scalar_mul(o[:], e[:], rc[:])
            nc.sync.dma_start(out=out_d[qc * 128:(qc + 1) * 128, :], in_=o[:])

    nc.compile()
    _cache["nc"] = nc
    return nc


def make_in_maps(h, adj, Wq, Wk, Wv, W1, b1, W2, b2):
    bf16 = ml_dtypes.bfloat16
    fp8 = ml_dtypes.float8_e4m3
    h32 = np.asarray(h, np.float32)
    ht8 = np.ascontiguousarray(
        h32.T.reshape(2, 128, N_NODES).transpose(1, 0, 2)).astype(fp8)

    def pack_w(W):
        # [r, dc, hd*64+f] = W[dc*128+r, hd, f]
        W = np.asarray(W, np.float32).transpose(1, 0, 2).reshape(IN, H * DH)
        return np.ascontiguousarray(
            W.reshape(2, 128, H * DH).transpose(1, 0, 2)).astype(fp8)

    wq8, wk8, wv8 = pack_w(Wq), pack_w(Wk), pack_w(Wv)
    W1b = np.asarray(W1, np.float32).astype(bf16)
    W2b = np.asarray(W2, np.float32).astype(bf16)
    b1r = np.ascontiguousarray(np.asarray(b1, np.float32).reshape(F1 // 128, 128).T)
    b2b = np.asarray(b2, np.float32).reshape(1, DOUT).astype(bf16)
    # adj [H, N, N] fp32 0/1 -> fp8 (exact), then per-core transposed slice
    adj8 = np.asarray(adj, np.float32).astype(fp8)
    adjT8 = np.ascontiguousarray(adj8.transpose(2, 0, 1))  # [j, hd, q_glob]
    in_maps = []
    for c in range(NCORES):
        q0 = c * QS
        adjt = np.ascontiguousarray(
            adjT8[:, :, q0:q0 + QS]).reshape(N_NODES, H * QS)
        in_maps.append({
            "adjt": adjt,
            "ht8": ht8,
            "htq8": np.ascontiguousarray(ht8[:, :, q0:q0 + QS]),
            "wq8": wq8, "wk8": wk8, "wv8": wv8,
            "w1": W1b, "w2": W2b, "b1": b1r, "b2b": b2b,
        })
    return in_maps


def kernel(h, adj, Wq, Wk, Wv, W1, b1, W2, b2):
    import os
    nc = build()
    from concourse.bass_utils import run_bass_kernel_spmd
    in_maps = make_in_maps(h, adj, Wq, Wk, Wv, W1, b1, W2, b2)
    trace = bool(os.environ.get("BASS_KERNEL_TRACE"))
    res = run_bass_kernel_spmd(nc, in_maps, list(range(NCORES)), trace=trace)
    if trace and res.exec_time_ns is not None:
        print(f"HW exec time: {res.exec_time_ns} ns")
        kernel.last_exec_time_ns = res.exec_time_ns
    out = np.concatenate([np.asarray(res.results[c]["out"]) for c in range(NCORES)],
                         axis=0)
    return out.astype(np.float32)


# revision 63
# speedup vs baseline: 1.1582x; 1.1582x over previous
"""Graph-transformer block on 8 Trainium2 NeuronCores.

Sharding: each core takes a 512-row q-slice of the 4096 nodes across ALL 4
heads. No cross-core communication: each core computes attention for its
q rows, runs the FFN on its node slice, writes its [512, 256] output slice.

Per-core pipeline (transposed-S orientation — no on-device transposes):
  prep:  q/k/v projections as fp8 DoubleRow matmuls (h and Wq/Wk/Wv ship
         as fp8; the IN=256 contraction runs as 2 k-tiles of 128), copied
         out of PSUM to bf16 qT/kT (head pairs packed on partitions) and
         to v with an interleaved all-ones column per head (the softmax
         denominator rides the PV matmul as output row 64). PSUM->SBUF
         copies go through the Scalar engine (ACT); GPSIMD cannot read
         PSUM. Prep chunks past j-block 7 are emitted at the END of
         attention loop bodies so they fill PE slack instead of blocking
         the S^T chain.
  attn:  per 128-wide j-block jb:
           one fp8 DMA of adjT for all 4 heads [128j, 4*512q]
           per head pair g: two S^T bf16 matmuls into one [128, 2, 512]
             PSUM tile (head 2g+i from partitions i*64 of packed kT/qT)
           mk = (S^T * 1/16) * adjT   (DVE STT, PSUM fp32 * fp8 -> bf16)
           P^T = exp(mk) on ACT, one [128, 2048] activation per j-block
           xaug[hd] [65, 512] += v_aug_blk.T @ P^T on PE (PSUM accum)
         PV matmuls lag the S^T stage by PIPE j-blocks so the in-order PE
         stream never waits on the DVE->ACT chain.
  fin:   embT = xaug[0:64] * recip(xaug[64]) broadcast -> [256, 512] bf16
  ffn:   p1^T = relu(W1.T @ embT + b1); p2 = p1 @ W2 (+ b2 via rank-1 ones
         matmul); row softmax over 256 features (logits are ~1e-3 scale so
         exp is safe without max-subtraction); DMA out fp32

Measured on hw: 131232 ns vs the 1368737 ns session baseline (10.4x).
Rejected by measurement (slower on hardware despite cost-model wins):
fp8 DoubleRow for the S^T or PV attention matmuls, fp8 S^T operands,
SBUF->SBUF DMA transposes of P (the original baseline's approach),
prep fully upfront, deeper PV lag.
"""
import sys
import numpy as np

sys.path.insert(0, "/opt/trn_rl_repo")
import ml_dtypes  # noqa: E402

IN = 256
H = 4
DH = 64
NCORES = 8
F1 = 512
DOUT = 256
N_NODES = 4096
QS = 512
NJB = N_NODES // 128
SCALE = 1.0 / 16.0  # 1/sqrt(IN)
PIPE = 2            # PV matmuls lag S^T by this many j-blocks

_cache = {}


def build():
    if "nc" in _cache:
        return _cache["nc"]

    from contextlib import ExitStack
    import concourse.tile as tile
    from concourse import mybir, bacc
    from concourse.alu_op_type import AluOpType

    fp32, bf16 = mybir.dt.float32, mybir.dt.bfloat16
    fp8 = mybir.dt.float8e4
    AF = mybir.ActivationFunctionType
    MUL = AluOpType.mult
    ADD = AluOpType.add

    nc = bacc.Bacc("TRN2", target_bir_lowering=False, debug=False,
                   enable_asserts=False)

    adjt_d = nc.dram_tensor("adjt", [N_NODES, H * QS], fp8, kind="ExternalInput").ap()
    ht8_d = nc.dram_tensor("ht8", [128, 2, N_NODES], fp8, kind="ExternalInput").ap()
    htq8_d = nc.dram_tensor("htq8", [128, 2, QS], fp8, kind="ExternalInput").ap()
    wq8_d = nc.dram_tensor("wq8", [128, 2, IN], fp8, kind="ExternalInput").ap()
    wk8_d = nc.dram_tensor("wk8", [128, 2, IN], fp8, kind="ExternalInput").ap()
    wv8_d = nc.dram_tensor("wv8", [128, 2, IN], fp8, kind="ExternalInput").ap()
    w1_d = nc.dram_tensor("w1", [IN, F1], bf16, kind="ExternalInput").ap()
    w2_d = nc.dram_tensor("w2", [F1, DOUT], bf16, kind="ExternalInput").ap()
    b1_d = nc.dram_tensor("b1", [128, F1 // 128], fp32, kind="ExternalInput").ap()
    b2b_d = nc.dram_tensor("b2b", [1, DOUT], bf16, kind="ExternalInput").ap()
    out_d = nc.dram_tensor("out", [QS, DOUT], fp32, kind="ExternalOutput").ap()

    with ExitStack() as ctx:
        tc = ctx.enter_context(tile.TileContext(nc))
        pc = ctx.enter_context(tc.tile_pool(name="const", bufs=1))
        pst = ctx.enter_context(tc.tile_pool(name="stp", bufs=2, space="PSUM"))
        pxt = ctx.enter_context(tc.tile_pool(name="xtp", bufs=1, space="PSUM"))
        pa = ctx.enter_context(tc.tile_pool(name="adjp", bufs=6))
        pm = ctx.enter_context(tc.tile_pool(name="mkp", bufs=5))
        ppt = ctx.enter_context(tc.tile_pool(name="ptp", bufs=4))
        psm = ctx.enter_context(tc.tile_pool(name="smallp", bufs=2))

        # ---------------- constant loads ----------------
        # hTq + weights + the first h j-chunk land first so the first prep
        # matmuls aren't gated on the full h transfer.
        htq8_sb = pc.tile([128, 2, QS], fp8, tag="htq8")
        nc.gpsimd.dma_start(out=htq8_sb[:, :, :], in_=htq8_d[:, :, :])
        wq8_sb = pc.tile([128, 2, IN], fp8, tag="wq8")
        wk8_sb = pc.tile([128, 2, IN], fp8, tag="wk8")
        wv8_sb = pc.tile([128, 2, IN], fp8, tag="wv8")
        for sb, dtsr in ((wq8_sb, wq8_d), (wk8_sb, wk8_d), (wv8_sb, wv8_d)):
            nc.gpsimd.dma_start(out=sb[:, :, :], in_=dtsr[:, :, :])
        ht8_sb = pc.tile([128, 2, N_NODES], fp8, tag="ht8")
        for jt in range(4):
            nc.gpsimd.dma_start(
                out=ht8_sb[:, :, jt * 1024:(jt + 1) * 1024],
                in_=ht8_d[:, :, jt * 1024:(jt + 1) * 1024])
        w1_sb = [pc.tile([128, F1], bf16, tag=f"w1_{dc}", name=f"w1_{dc}") for dc in range(2)]
        for dc in range(2):
            nc.gpsimd.dma_start(out=w1_sb[dc][:], in_=w1_d[dc * 128:(dc + 1) * 128, :])
        w2_sb = pc.tile([128, 4 * DOUT], bf16, tag="w2")
        for fc in range(4):
            nc.gpsimd.dma_start(out=w2_sb[:, fc * DOUT:(fc + 1) * DOUT],
                                in_=w2_d[fc * 128:(fc + 1) * 128, :])
        b1_sb = pc.tile([128, F1 // 128], fp32, tag="b1")
        nc.gpsimd.dma_start(out=b1_sb[:], in_=b1_d[:, :])
        b2b_sb = pc.tile([1, DOUT], bf16, tag="b2b")
        nc.gpsimd.dma_start(out=b2b_sb[:], in_=b2b_d[:, :])
        ones1_sb = pc.tile([1, 128], bf16, tag="ones1")
        nc.gpsimd.memset(ones1_sb[:], 1.0)

        # ---------------- projections ----------------
        # qT/kT bf16, head pairs packed on partitions (pair p -> head 2p at
        # partitions 0-63, 2p+1 at 64-127).
        qT_sb = [pc.tile([128, QS], bf16, tag=f"qT{p}", name=f"qT{p}") for p in range(2)]
        kT_sb = [pc.tile([128, N_NODES], bf16, tag=f"kT{p}", name=f"kT{p}") for p in range(2)]
        # v with interleaved ones columns: vp[:, jb*4+hd, 0:64] = v values for
        # head hd at j-block jb; vp[:, *, 64] = 1.0 (softmax denominator row)
        vp = pc.tile([128, NJB * H, DH + 1], bf16, tag="vp")
        nc.vector.memset(vp[:, :, DH:DH + 1], 1.0)

        DR = mybir.MatmulPerfMode.DoubleRow

        def emit_qT(p):
            ps = pst.tile([128, QS], fp32, tag="st", name=f"qTps{p}")
            nc.tensor.matmul(ps[:], wq8_sb[:, :, p * 128:(p + 1) * 128],
                             htq8_sb[:, :, :], start=True, stop=True,
                             perf_mode=DR)
            nc.scalar.copy(qT_sb[p][:], ps[:])

        def emit_kT(p, jt):  # one 1024-wide j chunk of kT for pair p
            ps = pst.tile([128, 2, 512], fp32, tag="st", name=f"kTps{p}_{jt}")
            for half in range(2):
                nc.tensor.matmul(
                    ps[:, half, :],
                    wk8_sb[:, :, p * 128:(p + 1) * 128],
                    ht8_sb[:, :, jt * 1024 + half * 512: jt * 1024 + (half + 1) * 512],
                    start=True, stop=True, perf_mode=DR)
            nc.scalar.copy(kT_sb[p][:, jt * 1024:(jt + 1) * 1024], ps[:, :, :])

        def emit_v(jq):  # v (with interleaved ones) for j-blocks 4jq..4jq+3
            ps = pst.tile([128, 16, DH], fp32, tag="st", name=f"vps{jq}")
            for jj in range(4):
                jb = jq * 4 + jj
                nc.tensor.matmul(ps[:, jj * 4:(jj + 1) * 4, :],
                                 ht8_sb[:, :, jb * 128:(jb + 1) * 128],
                                 wv8_sb[:, :, :],
                                 start=True, stop=True, perf_mode=DR)
            nc.scalar.copy(vp[:, jq * 16:(jq + 1) * 16, 0:DH], ps[:, :, :])

        emit_qT(0)
        emit_qT(1)
        emit_kT(0, 0)
        emit_kT(1, 0)
        emit_v(0)
        emit_v(1)
        # remaining chunks ordered by deadline: kT jt=t needed at jb 8t,
        # v quad jq needed at jb 4jq+2 (PV lags by PIPE).
        prep_chunks = []
        for t in range(1, 4):
            prep_chunks.append(lambda jt=t, p=0: emit_kT(p, jt))
            prep_chunks.append(lambda jt=t, p=1: emit_kT(p, jt))
            prep_chunks.append(lambda jq=2 * t: emit_v(jq))
            prep_chunks.append(lambda jq=2 * t + 1: emit_v(jq))

        # ---------------- attention ----------------
        embT_sb = [pc.tile([128, QS], bf16, tag=f"embT{p}", name=f"embT{p}") for p in range(2)]
        xaug = [pxt.tile([DH + 1, QS], fp32, tag=f"xt{hd}", name=f"xt{hd}") for hd in range(H)]

        pt_q = []
        for jb in range(NJB + PIPE):
            if jb < NJB:
                aj = pa.tile([128, H * QS], fp8, tag="aj")
                nc.sync.dma_start(out=aj[:],
                                  in_=adjt_d[jb * 128:(jb + 1) * 128, :])
                mk4 = pm.tile([128, H, QS], bf16, tag="mk")
                pt4 = ppt.tile([128, H, QS], bf16, tag="pt")
                for g in range(2):  # head pair
                    st2 = pst.tile([128, 2, 512], fp32, tag="st")
                    for i in range(2):  # head 2g+i from partitions i*64
                        nc.tensor.matmul(
                            st2[:, i, :],
                            kT_sb[g][i * 64:(i + 1) * 64, jb * 128:(jb + 1) * 128],
                            qT_sb[g][i * 64:(i + 1) * 64, :],
                            start=True, stop=True)
                    nc.vector.scalar_tensor_tensor(
                        mk4[:, 2 * g:2 * g + 2, :], st2[:, :, :], SCALE,
                        aj[:, g * 1024:(g + 1) * 1024], MUL, MUL)
                nc.scalar.activation(pt4[:, :, :], mk4[:, :, :], AF.Exp)
                pt_q.append((jb, pt4))
            if jb >= PIPE:
                j2, pt = pt_q.pop(0)
                for hd in range(H):
                    nc.tensor.matmul(xaug[hd][:],
                                     vp[:, j2 * H + hd, 0:DH + 1],
                                     pt[:, hd, :],
                                     start=(j2 == 0), stop=(j2 == NJB - 1))
            # prep chunks fill PE slack behind the attention chain
            if jb % 2 == 0 and prep_chunks:
                prep_chunks.pop(0)()

        # epilogue: embT = xaug[0:64] * (1/denom), denom = xaug row 64.
        # Stage-major emission so the four heads pipeline across DVE/Pool.
        dens = []
        for hd in range(H):
            den = psm.tile([1, QS], fp32, tag=f"den{hd}", name=f"den{hd}")
            nc.vector.tensor_copy(den[:], xaug[hd][DH:DH + 1, :])
            dens.append(den)
        rcps = []
        for hd in range(H):
            rcp = psm.tile([1, QS], fp32, tag=f"rcp{hd}", name=f"rcp{hd}")
            nc.vector.reciprocal_approx_fast(rcp[:], dens[hd][:])
            rcps.append(rcp)
        rbcs = []
        for hd in range(H):
            rbc = psm.tile([128, QS], fp32, tag=f"rbc{hd}", name=f"rbc{hd}")
            nc.gpsimd.partition_broadcast(rbc[:], rcps[hd][0:1, :])
            rbcs.append(rbc)
        for hd in range(H):
            p, off = hd // 2, (hd % 2) * 64
            nc.vector.tensor_tensor(embT_sb[p][off:off + 64, :],
                                    xaug[hd][0:DH, :], rbcs[hd][off:off + 64, :],
                                    MUL)

        # ---------------- FFN + row softmax ----------------
        p1_sb = pc.tile([128, F1 // 128, QS], bf16, tag="p1")
        for fc in range(F1 // 128):
            ps = pst.tile([128, QS], fp32, tag="st")
            for dc in range(2):
                nc.tensor.matmul(ps[:], w1_sb[dc][:, fc * 128:(fc + 1) * 128],
                                 embT_sb[dc][:], start=(dc == 0), stop=(dc == 1))
            nc.scalar.activation(p1_sb[:, fc, :], ps[:], AF.Relu,
                                 bias=b1_sb[:, fc:fc + 1])
        for qc in range(QS // 128):
            ps2 = pst.tile([128, DOUT], fp32, tag="st")
            for fc in range(F1 // 128):
                nc.tensor.matmul(ps2[:],
                                 p1_sb[:, fc, qc * 128:(qc + 1) * 128],
                                 w2_sb[:, fc * DOUT:(fc + 1) * DOUT],
                                 start=(fc == 0), stop=False)
            nc.tensor.matmul(ps2[:], ones1_sb[0:1, :], b2b_sb[0:1, :],
                             start=False, stop=True)
            # logits are ~1e-2 scale here, so exp() is overflow-safe without
            # the usual max-subtraction (softmax is shift-invariant).
            e = psm.tile([128, DOUT], fp32, tag="e")
            sm = psm.tile([128, 1], fp32, tag="sm")
            nc.scalar.activation(e[:], ps2[:], AF.Exp, accum_out=sm[:])
            rc = psm.tile([128, 1], fp32, tag="rc")
            nc.vector.reciprocal_approx_fast(rc[:], sm[:])
            o = psm.tile([128, DOUT], fp32, tag="o")
            nc.vector.tensor_scalar_mul(o[:], e[:], rc[:])
            nc.sync.dma_start(out=out_d[qc * 128:(qc + 1) * 128, :], in_=o[:])

    nc.compile()
    _cache["nc"] = nc
    return nc


def make_in_maps(h, adj, Wq, Wk, Wv, W1, b1, W2, b2):
    bf16 = ml_dtypes.bfloat16
    fp8 = ml_dtypes.float8_e4m3
    h32 = np.asarray(h, np.float32)
    ht8 = np.ascontiguousarray(
        h32.T.reshape(2, 128, N_NODES).transpose(1, 0, 2)).astype(fp8)

    def pack_w(W):
        # [r, dc, hd*64+f] = W[dc*128+r, hd, f]
        W = np.asarray(W, np.float32).transpose(1, 0, 2).reshape(IN, H * DH)
        return np.ascontiguousarray(
            W.reshape(2, 128, H * DH).transpose(1, 0, 2)).astype(fp8)

    wq8, wk8, wv8 = pack_w(Wq), pack_w(Wk), pack_w(Wv)
    W1b = np.asarray(W1, np.float32).astype(bf16)
    W2b = np.asarray(W2, np.float32).astype(bf16)
    b1r = np.ascontiguousarray(np.asarray(b1, np.float32).reshape(F1 // 128, 128).T)
    b2b = np.asarray(b2, np.float32).reshape(1, DOUT).astype(bf16)
    # adj [H, N, N] fp32 0/1 -> fp8 (exact), then per-core transposed slice
    adj8 = np.asarray(adj, np.float32).astype(fp8)
    adjT8 = np.ascontiguousarray(adj8.transpose(2, 0, 1))  # [j, hd, q_glob]
    in_maps = []
    for c in range(NCORES):
        q0 = c * QS
        adjt = np.ascontiguousarray(
            adjT8[:, :, q0:q0 + QS]).reshape(N_NODES, H * QS)
        in_maps.append({
            "adjt": adjt,
            "ht8": ht8,
            "htq8": np.ascontiguousarray(ht8[:, :, q0:q0 + QS]),
            "wq8": wq8, "wk8": wk8, "wv8": wv8,
            "w1": W1b, "w2": W2b, "b1": b1r, "b2b": b2b,
        })
    return in_maps


def kernel(h, adj, Wq, Wk, Wv, W1, b1, W2, b2):
    import os
    nc = build()
    from concourse.bass_utils import run_bass_kernel_spmd
    in_maps = make_in_maps(h, adj, Wq, Wk, Wv, W1, b1, W2, b2)
    trace = bool(os.environ.get("BASS_KERNEL_TRACE"))
    res = run_bass_kernel_spmd(nc, in_maps, list(range(NCORES)), trace=trace)
    if trace and res.exec_time_ns is not None:
        print(f"HW exec time: {res.exec_time_ns} ns")
        kernel.last_exec_time_ns = res.exec_time_ns
    out = np.concatenate([np.asarray(res.results[c]["out"]) for c in range(NCORES)],
                         axis=0)
    return out.astype(np.float32)


# revision 64
# speedup vs baseline: 1.1663x; 1.0071x over previous
"""Graph-transformer block on 8 Trainium2 NeuronCores.

Sharding: each core takes a 512-row q-slice of the 4096 nodes across ALL 4
heads. No cross-core communication: each core computes attention for its
q rows, runs the FFN on its node slice, writes its [512, 256] output slice.

Per-core pipeline (transposed-S orientation — no on-device transposes):
  prep:  q/k/v projections as fp8 DoubleRow matmuls (h and Wq/Wk/Wv ship
         as fp8; the IN=256 contraction runs as 2 k-tiles of 128), copied
         out of PSUM to bf16 qT/kT (head pairs packed on partitions) and
         to v with an interleaved all-ones column per head (the softmax
         denominator rides the PV matmul as output row 64). PSUM->SBUF
         copies go through the Scalar engine (ACT); GPSIMD cannot read
         PSUM. Prep chunks past j-block 7 are emitted at the END of
         attention loop bodies so they fill PE slack instead of blocking
         the S^T chain.
  attn:  per 128-wide j-block jb:
           one fp8 DMA of adjT for all 4 heads [128j, 4*512q]
           per head pair g: two S^T bf16 matmuls into one [128, 2, 512]
             PSUM tile (head 2g+i from partitions i*64 of packed kT/qT)
           mk = (S^T * 1/16) * adjT   (DVE STT, PSUM fp32 * fp8 -> bf16)
           P^T = exp(mk) on ACT, one [128, 2048] activation per j-block
           xaug[hd] [65, 512] += v_aug_blk.T @ P^T on PE (PSUM accum)
         PV matmuls lag the S^T stage by PIPE j-blocks so the in-order PE
         stream never waits on the DVE->ACT chain.
  fin:   embT = xaug[0:64] * recip(xaug[64]) broadcast -> [256, 512] bf16
  ffn:   p1^T = relu(W1.T @ embT + b1); p2 = p1 @ W2 (+ b2 via rank-1 ones
         matmul); row softmax over 256 features (logits are ~1e-3 scale so
         exp is safe without max-subtraction); DMA out fp32

Measured on hw: 131232 ns vs the 1368737 ns session baseline (10.4x).
Rejected by measurement (slower on hardware despite cost-model wins):
fp8 DoubleRow for the S^T or PV attention matmuls, fp8 S^T operands,
SBUF->SBUF DMA transposes of P (the original baseline's approach),
prep fully upfront, deeper PV lag.
"""
import sys
import numpy as np

sys.path.insert(0, "/opt/trn_rl_repo")
import ml_dtypes  # noqa: E402

IN = 256
H = 4
DH = 64
NCORES = 8
F1 = 512
DOUT = 256
N_NODES = 4096
QS = 512
NJB = N_NODES // 128
SCALE = 1.0 / 16.0  # 1/sqrt(IN)
PIPE = 2            # PV matmuls lag S^T by this many j-blocks

_cache = {}


def build():
    if "nc" in _cache:
        return _cache["nc"]

    from contextlib import ExitStack
    import concourse.tile as tile
    from concourse import mybir, bacc
    from concourse.alu_op_type import AluOpType

    fp32, bf16 = mybir.dt.float32, mybir.dt.bfloat16
    fp8 = mybir.dt.float8e4
    AF = mybir.ActivationFunctionType
    MUL = AluOpType.mult
    ADD = AluOpType.add

    nc = bacc.Bacc("TRN2", target_bir_lowering=False, debug=False,
                   enable_asserts=False)

    adjt_d = nc.dram_tensor("adjt", [N_NODES, H * QS], fp8, kind="ExternalInput").ap()
    ht8_d = nc.dram_tensor("ht8", [128, 2, N_NODES], fp8, kind="ExternalInput").ap()
    htq8_d = nc.dram_tensor("htq8", [128, 2, QS], fp8, kind="ExternalInput").ap()
    wq8_d = nc.dram_tensor("wq8", [128, 2, IN], fp8, kind="ExternalInput").ap()
    wk8_d = nc.dram_tensor("wk8", [128, 2, IN], fp8, kind="ExternalInput").ap()
    wv8_d = nc.dram_tensor("wv8", [128, 2, IN], fp8, kind="ExternalInput").ap()
    w1_d = nc.dram_tensor("w1", [IN, F1], bf16, kind="ExternalInput").ap()
    w2_d = nc.dram_tensor("w2", [F1, DOUT], bf16, kind="ExternalInput").ap()
    b1_d = nc.dram_tensor("b1", [128, F1 // 128], fp32, kind="ExternalInput").ap()
    b2b_d = nc.dram_tensor("b2b", [1, DOUT], bf16, kind="ExternalInput").ap()
    out_d = nc.dram_tensor("out", [QS, DOUT], fp32, kind="ExternalOutput").ap()

    with ExitStack() as ctx:
        tc = ctx.enter_context(tile.TileContext(nc))
        pc = ctx.enter_context(tc.tile_pool(name="const", bufs=1))
        pst = ctx.enter_context(tc.tile_pool(name="stp", bufs=2, space="PSUM"))
        pxt = ctx.enter_context(tc.tile_pool(name="xtp", bufs=1, space="PSUM"))
        pa = ctx.enter_context(tc.tile_pool(name="adjp", bufs=6))
        pm = ctx.enter_context(tc.tile_pool(name="mkp", bufs=6))
        ppt = ctx.enter_context(tc.tile_pool(name="ptp", bufs=6))
        psm = ctx.enter_context(tc.tile_pool(name="smallp", bufs=2))

        # ---------------- constant loads ----------------
        # hTq + weights + the first h j-chunk land first so the first prep
        # matmuls aren't gated on the full h transfer.
        htq8_sb = pc.tile([128, 2, QS], fp8, tag="htq8")
        nc.gpsimd.dma_start(out=htq8_sb[:, :, :], in_=htq8_d[:, :, :])
        wq8_sb = pc.tile([128, 2, IN], fp8, tag="wq8")
        wk8_sb = pc.tile([128, 2, IN], fp8, tag="wk8")
        wv8_sb = pc.tile([128, 2, IN], fp8, tag="wv8")
        for sb, dtsr in ((wq8_sb, wq8_d), (wk8_sb, wk8_d), (wv8_sb, wv8_d)):
            nc.gpsimd.dma_start(out=sb[:, :, :], in_=dtsr[:, :, :])
        ht8_sb = pc.tile([128, 2, N_NODES], fp8, tag="ht8")
        for jt in range(4):
            nc.gpsimd.dma_start(
                out=ht8_sb[:, :, jt * 1024:(jt + 1) * 1024],
                in_=ht8_d[:, :, jt * 1024:(jt + 1) * 1024])
        w1_sb = [pc.tile([128, F1], bf16, tag=f"w1_{dc}", name=f"w1_{dc}") for dc in range(2)]
        for dc in range(2):
            nc.gpsimd.dma_start(out=w1_sb[dc][:], in_=w1_d[dc * 128:(dc + 1) * 128, :])
        w2_sb = pc.tile([128, 4 * DOUT], bf16, tag="w2")
        for fc in range(4):
            nc.gpsimd.dma_start(out=w2_sb[:, fc * DOUT:(fc + 1) * DOUT],
                                in_=w2_d[fc * 128:(fc + 1) * 128, :])
        b1_sb = pc.tile([128, F1 // 128], fp32, tag="b1")
        nc.gpsimd.dma_start(out=b1_sb[:], in_=b1_d[:, :])
        b2b_sb = pc.tile([1, DOUT], bf16, tag="b2b")
        nc.gpsimd.dma_start(out=b2b_sb[:], in_=b2b_d[:, :])
        ones1_sb = pc.tile([1, 128], bf16, tag="ones1")
        nc.gpsimd.memset(ones1_sb[:], 1.0)

        # ---------------- projections ----------------
        # qT/kT bf16, head pairs packed on partitions (pair p -> head 2p at
        # partitions 0-63, 2p+1 at 64-127).
        qT_sb = [pc.tile([128, QS], bf16, tag=f"qT{p}", name=f"qT{p}") for p in range(2)]
        kT_sb = [pc.tile([128, N_NODES], bf16, tag=f"kT{p}", name=f"kT{p}") for p in range(2)]
        # v with interleaved ones columns: vp[:, jb*4+hd, 0:64] = v values for
        # head hd at j-block jb; vp[:, *, 64] = 1.0 (softmax denominator row)
        vp = pc.tile([128, NJB * H, DH + 1], bf16, tag="vp")
        nc.vector.memset(vp[:, :, DH:DH + 1], 1.0)

        DR = mybir.MatmulPerfMode.DoubleRow

        def emit_qT(p):
            ps = pst.tile([128, QS], fp32, tag="st", name=f"qTps{p}")
            nc.tensor.matmul(ps[:], wq8_sb[:, :, p * 128:(p + 1) * 128],
                             htq8_sb[:, :, :], start=True, stop=True,
                             perf_mode=DR)
            nc.scalar.copy(qT_sb[p][:], ps[:])

        def emit_kT(p, jt):  # one 1024-wide j chunk of kT for pair p
            ps = pst.tile([128, 2, 512], fp32, tag="st", name=f"kTps{p}_{jt}")
            for half in range(2):
                nc.tensor.matmul(
                    ps[:, half, :],
                    wk8_sb[:, :, p * 128:(p + 1) * 128],
                    ht8_sb[:, :, jt * 1024 + half * 512: jt * 1024 + (half + 1) * 512],
                    start=True, stop=True, perf_mode=DR)
            nc.scalar.copy(kT_sb[p][:, jt * 1024:(jt + 1) * 1024], ps[:, :, :])

        def emit_v(jq):  # v (with interleaved ones) for j-blocks 4jq..4jq+3
            ps = pst.tile([128, 16, DH], fp32, tag="st", name=f"vps{jq}")
            for jj in range(4):
                jb = jq * 4 + jj
                nc.tensor.matmul(ps[:, jj * 4:(jj + 1) * 4, :],
                                 ht8_sb[:, :, jb * 128:(jb + 1) * 128],
                                 wv8_sb[:, :, :],
                                 start=True, stop=True, perf_mode=DR)
            nc.scalar.copy(vp[:, jq * 16:(jq + 1) * 16, 0:DH], ps[:, :, :])

        emit_qT(0)
        emit_qT(1)
        emit_kT(0, 0)
        emit_kT(1, 0)
        emit_v(0)
        emit_v(1)
        # remaining chunks ordered by deadline: kT jt=t needed at jb 8t,
        # v quad jq needed at jb 4jq+2 (PV lags by PIPE).
        prep_chunks = []
        for t in range(1, 4):
            prep_chunks.append(lambda jt=t, p=0: emit_kT(p, jt))
            prep_chunks.append(lambda jt=t, p=1: emit_kT(p, jt))
            prep_chunks.append(lambda jq=2 * t: emit_v(jq))
            prep_chunks.append(lambda jq=2 * t + 1: emit_v(jq))

        # ---------------- attention ----------------
        embT_sb = [pc.tile([128, QS], bf16, tag=f"embT{p}", name=f"embT{p}") for p in range(2)]
        xaug = [pxt.tile([DH + 1, QS], fp32, tag=f"xt{hd}", name=f"xt{hd}") for hd in range(H)]

        pt_q = []
        for jb in range(NJB + PIPE):
            if jb < NJB:
                aj = pa.tile([128, H * QS], fp8, tag="aj")
                nc.sync.dma_start(out=aj[:],
                                  in_=adjt_d[jb * 128:(jb + 1) * 128, :])
                mk4 = pm.tile([128, H, QS], bf16, tag="mk")
                pt4 = ppt.tile([128, H, QS], bf16, tag="pt")
                for g in range(2):  # head pair
                    st2 = pst.tile([128, 2, 512], fp32, tag="st")
                    for i in range(2):  # head 2g+i from partitions i*64
                        nc.tensor.matmul(
                            st2[:, i, :],
                            kT_sb[g][i * 64:(i + 1) * 64, jb * 128:(jb + 1) * 128],
                            qT_sb[g][i * 64:(i + 1) * 64, :],
                            start=True, stop=True)
                    nc.vector.scalar_tensor_tensor(
                        mk4[:, 2 * g:2 * g + 2, :], st2[:, :, :], SCALE,
                        aj[:, g * 1024:(g + 1) * 1024], MUL, MUL)
                nc.scalar.activation(pt4[:, :, :], mk4[:, :, :], AF.Exp)
                pt_q.append((jb, pt4))
            if jb >= PIPE:
                j2, pt = pt_q.pop(0)
                for hd in range(H):
                    nc.tensor.matmul(xaug[hd][:],
                                     vp[:, j2 * H + hd, 0:DH + 1],
                                     pt[:, hd, :],
                                     start=(j2 == 0), stop=(j2 == NJB - 1))
            # prep chunks fill PE slack behind the attention chain
            if jb % 2 == 0 and prep_chunks:
                prep_chunks.pop(0)()

        # epilogue: embT = xaug[0:64] * (1/denom), denom = xaug row 64.
        # Stage-major emission so the four heads pipeline across DVE/Pool.
        dens = []
        for hd in range(H):
            den = psm.tile([1, QS], fp32, tag=f"den{hd}", name=f"den{hd}")
            nc.vector.tensor_copy(den[:], xaug[hd][DH:DH + 1, :])
            dens.append(den)
        rcps = []
        for hd in range(H):
            rcp = psm.tile([1, QS], fp32, tag=f"rcp{hd}", name=f"rcp{hd}")
            nc.vector.reciprocal_approx_fast(rcp[:], dens[hd][:])
            rcps.append(rcp)
        rbcs = []
        for hd in range(H):
            rbc = psm.tile([128, QS], fp32, tag=f"rbc{hd}", name=f"rbc{hd}")
            nc.gpsimd.partition_broadcast(rbc[:], rcps[hd][0:1, :])
            rbcs.append(rbc)
        for hd in range(H):
            p, off = hd // 2, (hd % 2) * 64
            nc.vector.tensor_tensor(embT_sb[p][off:off + 64, :],
                                    xaug[hd][0:DH, :], rbcs[hd][off:off + 64, :],
                                    MUL)

        # ---------------- FFN + row softmax ----------------
        p1_sb = pc.tile([128, F1 // 128, QS], bf16, tag="p1")
        for fc in range(F1 // 128):
            ps = pst.tile([128, QS], fp32, tag="st")
            for dc in range(2):
                nc.tensor.matmul(ps[:], w1_sb[dc][:, fc * 128:(fc + 1) * 128],
                                 embT_sb[dc][:], start=(dc == 0), stop=(dc == 1))
            nc.scalar.activation(p1_sb[:, fc, :], ps[:], AF.Relu,
                                 bias=b1_sb[:, fc:fc + 1])
        for qc in range(QS // 128):
            ps2 = pst.tile([128, DOUT], fp32, tag="st")
            for fc in range(F1 // 128):
                nc.tensor.matmul(ps2[:],
                                 p1_sb[:, fc, qc * 128:(qc + 1) * 128],
                                 w2_sb[:, fc * DOUT:(fc + 1) * DOUT],
                                 start=(fc == 0), stop=False)
            nc.tensor.matmul(ps2[:], ones1_sb[0:1, :], b2b_sb[0:1, :],
                             start=False, stop=True)
            # logits are ~1e-2 scale here, so exp() is overflow-safe without
            # the usual max-subtraction (softmax is shift-invariant).
            e = psm.tile([128, DOUT], fp32, tag="e")
            sm = psm.tile([128, 1], fp32, tag="sm")
            nc.scalar.activation(e[:], ps2[:], AF.Exp, accum_out=sm[:])
            rc = psm.tile([128, 1], fp32, tag="rc")
            nc.vector.reciprocal_approx_fast(rc[:], sm[:])
            o = psm.tile([128, DOUT], fp32, tag="o")
            nc.vector.tensor_scalar_mul(o[:], e[:], rc[:])
            nc.sync.dma_start(out=out_d[qc * 128:(qc + 1) * 128, :], in_=o[:])

    nc.compile()
    _cache["nc"] = nc
    return nc


def make_in_maps(h, adj, Wq, Wk, Wv, W1, b1, W2, b2):
    bf16 = ml_dtypes.bfloat16
    fp8 = ml_dtypes.float8_e4m3
    h32 = np.asarray(h, np.float32)
    ht8 = np.ascontiguousarray(
        h32.T.reshape(2, 128, N_NODES).transpose(1, 0, 2)).astype(fp8)

    def pack_w(W):
        # [r, dc, hd*64+f] = W[dc*128+r, hd, f]
        W = np.asarray(W, np.float32).transpose(1, 0, 2).reshape(IN, H * DH)
        return np.ascontiguousarray(
            W.reshape(2, 128, H * DH).transpose(1, 0, 2)).astype(fp8)

    wq8, wk8, wv8 = pack_w(Wq), pack_w(Wk), pack_w(Wv)
    W1b = np.asarray(W1, np.float32).astype(bf16)
    W2b = np.asarray(W2, np.float32).astype(bf16)
    b1r = np.ascontiguousarray(np.asarray(b1, np.float32).reshape(F1 // 128, 128).T)
    b2b = np.asarray(b2, np.float32).reshape(1, DOUT).astype(bf16)
    # adj [H, N, N] fp32 0/1 -> fp8 (exact), then per-core transposed slice
    adj8 = np.asarray(adj, np.float32).astype(fp8)
    adjT8 = np.ascontiguousarray(adj8.transpose(2, 0, 1))  # [j, hd, q_glob]
    in_maps = []
    for c in range(NCORES):
        q0 = c * QS
        adjt = np.ascontiguousarray(
            adjT8[:, :, q0:q0 + QS]).reshape(N_NODES, H * QS)
        in_maps.append({
            "adjt": adjt,
            "ht8": ht8,
            "htq8": np.ascontiguousarray(ht8[:, :, q0:q0 + QS]),
            "wq8": wq8, "wk8": wk8, "wv8": wv8,
            "w1": W1b, "w2": W2b, "b1": b1r, "b2b": b2b,
        })
    return in_maps


def kernel(h, adj, Wq, Wk, Wv, W1, b1, W2, b2):
    import os
    nc = build()
    from concourse.bass_utils import run_bass_kernel_spmd
    in_maps = make_in_maps(h, adj, Wq, Wk, Wv, W1, b1, W2, b2)
    trace = bool(os.environ.get("BASS_KERNEL_TRACE"))
    res = run_bass_kernel_spmd(nc, in_maps, list(range(NCORES)), trace=trace)
    if trace and res.exec_time_ns is not None:
        print(f"HW exec time: {res.exec_time_ns} ns")
        kernel.last_exec_time_ns = res.exec_time_ns
    out = np.concatenate([np.asarray(res.results[c]["out"]) for c in range(NCORES)],
                         axis=0)
    return out.astype(np.float32)
